# revision 5
# baseline (speedup 1.0000x reference)
# Trainium2 Bass kernel for nn_AttentionBlock (GroupNorm + single-head
# self-attention over 32x32 spatial, C=512) — data-parallel over batch:
# 8 batch elements -> 8 NeuronCores, weights replicated.
#
# Self-contained: builds the Bass module lazily, shards the full inputs,
# runs via concourse.bass_utils.run_bass_kernel_spmd, gathers the output.
import numpy as np

CH = 512          # channels
N = 1024          # spatial H*W = 32*32
P = 128           # SBUF partitions
KT = CH // P      # 4 channel tiles
MT = N // P       # 8 spatial tiles (keys)
GROUPS = 8        # groupnorm groups (64 channels each)
EPS = 1e-5
SCALE = 1.0 / np.sqrt(CH)
NCORES = 8

# Fold softmax 1/sum into the PE transpose by replacing the identity
# moving operand with diag(1/sum). Falls back to an explicit DVE
# normalization pass when False.
USE_DIAG_FOLD = False
# Subtract the row max before exp (matches reference jax.nn.softmax).
USE_MAX_SUB = True

_CACHE = {}


def _build_bass():
    import concourse.bacc as bacc
    import concourse.tile as tile
    from concourse import mybir

    f32 = mybir.dt.float32
    f32r = mybir.dt.float32r
    Act = mybir.ActivationFunctionType
    Alu = mybir.AluOpType

    nc = bacc.Bacc("TRN2")

    x_d = nc.dram_tensor("x", [CH, N], f32, kind="ExternalInput")
    wq_d = nc.dram_tensor("wq_t", [CH, CH], f32, kind="ExternalInput")
    wk_d = nc.dram_tensor("wk_t", [CH, CH], f32, kind="ExternalInput")
    wv_d = nc.dram_tensor("wv_t", [CH, CH], f32, kind="ExternalInput")
    wp_d = nc.dram_tensor("wp_t", [CH, CH], f32, kind="ExternalInput")
    bq_d = nc.dram_tensor("bq_pc", [P, KT], f32, kind="ExternalInput")
    bk_d = nc.dram_tensor("bk_pc", [P, KT], f32, kind="ExternalInput")
    bv_d = nc.dram_tensor("bv_pc", [P, KT], f32, kind="ExternalInput")
    bp_d = nc.dram_tensor("bp_pc", [P, KT], f32, kind="ExternalInput")
    gnw_d = nc.dram_tensor("gnw_pc", [P, KT], f32, kind="ExternalInput")
    gnb_d = nc.dram_tensor("gnb_pc", [P, KT], f32, kind="ExternalInput")
    ident_d = nc.dram_tensor("ident", [P, P], f32, kind="ExternalInput")
    gmask_d = nc.dram_tensor("gmask", [P, 2], f32, kind="ExternalInput")
    bmask_d = nc.dram_tensor("bmask", [2, P], f32, kind="ExternalInput")
    y_d = nc.dram_tensor("y", [CH, N], f32, kind="ExternalOutput")

    def r(ap):
        return ap.bitcast(f32r)

    with tile.TileContext(nc) as tc:
        with (
            tc.tile_pool(name="persist", bufs=1) as persist,
            tc.tile_pool(name="work", bufs=2) as work,
            tc.tile_pool(name="small", bufs=2) as small,
            tc.tile_pool(name="ytiles", bufs=3) as ypool,
            tc.tile_pool(name="ps_s", bufs=2, space="PSUM") as ps_s,
            tc.tile_pool(name="ps_t", bufs=2, space="PSUM") as ps_t,
            tc.tile_pool(name="ps_mm", bufs=2, space="PSUM") as ps_mm,
        ):
            # ---- persistent SBUF tensors ----
            x_sb = persist.tile([P, KT, N], f32, tag="x")
            n_sb = persist.tile([P, KT, N], f32r, tag="n")
            q_sb = persist.tile([P, KT, N], f32r, tag="q")
            k_sb = persist.tile([P, KT, N], f32r, tag="k")
            vT_sb = persist.tile([P, MT, CH], f32r, tag="vT")
            aT_sb = persist.tile([P, MT, N], f32r, tag="aT")
            o_sb = persist.tile([P, KT, N], f32r, tag="o")
            wq_sb = persist.tile([P, KT, CH], f32r, tag="wq")
            wk_sb = persist.tile([P, KT, CH], f32r, tag="wk")
            wv_sb = persist.tile([P, KT, CH], f32r, tag="wv")
            wp_sb = persist.tile([P, KT, CH], f32r, tag="wp")
            bq_sb = persist.tile([P, KT], f32, tag="bq")
            bk_sb = persist.tile([P, KT], f32, tag="bk")
            bv_sb = persist.tile([P, KT], f32, tag="bv")
            bp_sb = persist.tile([P, KT], f32, tag="bp")
            gnw_sb = persist.tile([P, KT], f32, tag="gnw")
            gnb_sb = persist.tile([P, KT], f32, tag="gnb")
            ident_sb = persist.tile([P, P], f32, tag="ident")
            gmask_sb = persist.tile([P, 2], f32, tag="gmask")
            bmask_sb = persist.tile([2, P], f32, tag="bmask")

            # ---- loads ----
            for kt in range(KT):
                nc.sync.dma_start(out=x_sb[:, kt, :], in_=x_d[kt * P:(kt + 1) * P, :])
            for w_sb, w_d in ((wq_sb, wq_d), (wk_sb, wk_d), (wv_sb, wv_d), (wp_sb, wp_d)):
                for kt in range(KT):
                    nc.sync.dma_start(out=w_sb[:, kt, :], in_=w_d[kt * P:(kt + 1) * P, :].bitcast(f32r))
            for v_sb, v_d in (
                (bq_sb, bq_d), (bk_sb, bk_d), (bv_sb, bv_d), (bp_sb, bp_d),
                (gnw_sb, gnw_d), (gnb_sb, gnb_d), (ident_sb, ident_d),
                (gmask_sb, gmask_d), (bmask_sb, bmask_d),
            ):
                nc.sync.dma_start(out=v_sb[:], in_=v_d[:])

            # ---- GroupNorm ----
            # Per-channel mean/var via bn_stats, then 64-channel group
            # aggregation / broadcast via two tiny mask matmuls.
            statsin = persist.tile([P, 8], f32, tag="statsin")  # cols 0:4 mean, 4:8 E[x^2]
            for kt in range(KT):
                bstats = small.tile([P, 2, 6], f32, tag="bstats")
                mv = small.tile([P, 2], f32, tag="mv")
                nc.vector.bn_stats(out=bstats[:, 0, :], in_=x_sb[:, kt, 0:512])
                nc.vector.bn_stats(out=bstats[:, 1, :], in_=x_sb[:, kt, 512:1024])
                nc.vector.bn_aggr(out=mv, in_=bstats)
                nc.vector.tensor_copy(statsin[:, kt:kt + 1], mv[:, 0:1])
                # E[x^2] = var + mean^2
                nc.vector.scalar_tensor_tensor(
                    out=statsin[:, 4 + kt:5 + kt], in0=mv[:, 0:1], scalar=mv[:, 0:1],
                    in1=mv[:, 1:2], op0=Alu.mult, op1=Alu.add,
                )
            gs_ps = ps_mm.tile([2, 8], f32, tag="mm")
            nc.tensor.matmul(gs_ps, gmask_sb, statsin, start=True, stop=True)
            gstats = persist.tile([2, 8], f32, tag="gstats")
            nc.scalar.copy(gstats, gs_ps)
            bc_ps = ps_mm.tile([P, 8], f32, tag="mm")
            nc.tensor.matmul(bc_ps, bmask_sb, gstats, start=True, stop=True)
            bc_sb = persist.tile([P, 8], f32, tag="bc")
            nc.scalar.copy(bc_sb, bc_ps)

            mean_pc = bc_sb[:, 0:4]
            sq = small.tile([P, 4], f32, tag="sq")
            var_pc = small.tile([P, 4], f32, tag="var")
            nc.vector.tensor_mul(sq, mean_pc, mean_pc)
            nc.vector.tensor_sub(var_pc, bc_sb[:, 4:8], sq)  # var = E[x^2] - mean^2
            # t = var + eps ; rstd = 1/sqrt(t) with one Newton-Raphson step
            # (ACT Sqrt has a loose precision budget).
            tve = small.tile([P, 4], f32, tag="tve")
            nc.vector.tensor_scalar_add(tve, var_pc, EPS)
            eps_sb = persist.tile([P, 1], f32, tag="eps")
            nc.vector.memset(eps_sb, EPS)
            sd = small.tile([P, 4], f32, tag="sd")
            nc.scalar.activation(out=sd, in_=var_pc, func=Act.Sqrt, bias=eps_sb, scale=1.0)
            r0 = small.tile([P, 4], f32, tag="r0")
            nc.vector.reciprocal(r0, sd)
            rr = small.tile([P, 4], f32, tag="rr")
            nc.vector.tensor_mul(rr, r0, r0)           # r0^2
            nc.vector.tensor_mul(rr, tve, rr)          # t*r0^2
            nc.vector.tensor_scalar(rr, rr, -0.5, 1.5, op0=Alu.mult, op1=Alu.add)
            rstd = small.tile([P, 4], f32, tag="rstd")
            nc.vector.tensor_mul(rstd, r0, rr)         # r0*(1.5 - 0.5*t*r0^2)

            scale_pc = persist.tile([P, 4], f32, tag="scale_pc")
            shift_pc = persist.tile([P, 4], f32, tag="shift_pc")
            nc.vector.tensor_mul(scale_pc, rstd, gnw_sb)
            tmp2 = small.tile([P, 4], f32, tag="tmp2")
            nc.vector.tensor_mul(tmp2, mean_pc, scale_pc)
            nc.vector.tensor_sub(shift_pc, gnb_sb, tmp2)
            for kt in range(KT):
                nc.vector.tensor_scalar(
                    out=n_sb[:, kt, :], in0=x_sb[:, kt, :],
                    scalar1=scale_pc[:, kt:kt + 1], scalar2=shift_pc[:, kt:kt + 1],
                    op0=Alu.mult, op1=Alu.add,
                )

            # ---- Q, K projections: q[d, n] = sum_c wq[d, c] n[c, n] + bq[d] ----
            for w_sb, b_sb, dst in ((wq_sb, bq_sb, q_sb), (wk_sb, bk_sb, k_sb)):
                for dt in range(KT):
                    for nh in range(2):
                        mm = ps_mm.tile([P, 512], f32, tag="mm")
                        for kt in range(KT):
                            nc.tensor.matmul(
                                mm,
                                r(w_sb[:, kt, dt * P:(dt + 1) * P]),
                                n_sb[:, kt, nh * 512:(nh + 1) * 512],
                                start=(kt == 0), stop=(kt == KT - 1),
                            )
                        nc.scalar.activation(
                            out=dst[:, dt, nh * 512:(nh + 1) * 512], in_=mm,
                            func=Act.Identity, bias=b_sb[:, dt:dt + 1], scale=1.0,
                        )

            # ---- V transposed: vT[m, c] = sum_c' n[c', m] wv_t[c', c] ----
            # (v bias folds into the attention output: rows of attn sum to 1.)
            for mt in range(MT):
                mm = ps_mm.tile([P, 512], f32, tag="mm")
                for kt in range(KT):
                    nc.tensor.matmul(
                        mm,
                        n_sb[:, kt, mt * P:(mt + 1) * P],
                        wv_sb[:, kt, :],
                        start=(kt == 0), stop=(kt == KT - 1),
                    )
                nc.scalar.copy(vT_sb[:, mt, :], mm)

            # ---- attention per query block (128 queries) ----
            for nb in range(MT):
                s_ps = ps_s.tile([P, N], f32, tag="s")
                for mh in range(2):
                    for kt in range(KT):
                        nc.tensor.matmul(
                            s_ps[:, mh * 512:(mh + 1) * 512],
                            q_sb[:, kt, nb * P:(nb + 1) * P],
                            k_sb[:, kt, mh * 512:(mh + 1) * 512],
                            start=(kt == 0), stop=(kt == KT - 1),
                        )
                p_exp = work.tile([P, N], f32, tag="pexp")
                sumexp = small.tile([P, 1], f32, tag="sumexp")
                if USE_MAX_SUB:
                    smax = small.tile([P, 1], f32, tag="smax")
                    nc.vector.reduce_max(smax, s_ps, axis=mybir.AxisListType.X)
                    nmax = small.tile([P, 1], f32, tag="nmax")
                    nc.vector.tensor_scalar_mul(nmax, smax, -SCALE)
                    nc.scalar.activation(
                        out=p_exp, in_=s_ps, func=Act.Exp,
                        bias=nmax, scale=SCALE, accum_out=sumexp,
                    )
                else:
                    nc.scalar.activation(
                        out=p_exp, in_=s_ps, func=Act.Exp,
                        bias=0.0, scale=SCALE, accum_out=sumexp,
                    )
                rsum = small.tile([P, 1], f32, tag="rsum")
                nc.vector.reciprocal(rsum, sumexp)
                if USE_DIAG_FOLD:
                    # diag(1/sum): identity has a single 1 per row at col p;
                    # per-partition scalar multiply scales row p by r[p].
                    diag = work.tile([P, P], f32, tag="diag")
                    nc.vector.tensor_scalar_mul(diag, ident_sb, rsum)
                    tr_rhs = diag
                else:
                    nc.vector.tensor_scalar_mul(p_exp, p_exp, rsum)
                    tr_rhs = ident_sb
                # transpose 128x128 blocks: aT[:, mt, nb*128:...] = attn.T
                for mg in range(2):
                    t_ps = ps_t.tile([P, 512], f32, tag="t")
                    for mi in range(4):
                        mt = mg * 4 + mi
                        nc.tensor.transpose(
                            t_ps[:, mi * P:(mi + 1) * P],
                            p_exp[:, mt * P:(mt + 1) * P],
                            tr_rhs,
                        )
                    nc.vector.tensor_copy(
                        aT_sb[:, mg * 4:(mg + 1) * 4, nb * P:(nb + 1) * P],
                        t_ps.rearrange("p (g c) -> p g c", g=4),
                    )

            # ---- out[c, n] = sum_m vT[m, c] attnT[m, n] (+ bv, folded) ----
            for ct in range(KT):
                for nh in range(2):
                    mm = ps_mm.tile([P, 512], f32, tag="mm")
                    for mt in range(MT):
                        nc.tensor.matmul(
                            mm,
                            vT_sb[:, mt, ct * P:(ct + 1) * P],
                            aT_sb[:, mt, nh * 512:(nh + 1) * 512],
                            start=(mt == 0), stop=(mt == MT - 1),
                        )
                    nc.scalar.activation(
                        out=o_sb[:, ct, nh * 512:(nh + 1) * 512], in_=mm,
                        func=Act.Identity, bias=bv_sb[:, ct:ct + 1], scale=1.0,
                    )

            # ---- final projection + bias + residual, stream out ----
            for dt in range(KT):
                for nh in range(2):
                    mm = ps_mm.tile([P, 512], f32, tag="mm")
                    for kt in range(KT):
                        nc.tensor.matmul(
                            mm,
                            wp_sb[:, kt, dt * P:(dt + 1) * P],
                            o_sb[:, kt, nh * 512:(nh + 1) * 512],
                            start=(kt == 0), stop=(kt == KT - 1),
                        )
                    y_sb = ypool.tile([P, 512], f32, tag="y")
                    nc.vector.scalar_tensor_tensor(
                        out=y_sb, in0=mm, scalar=bp_sb[:, dt:dt + 1],
                        in1=x_sb[:, dt, nh * 512:(nh + 1) * 512],
                        op0=Alu.add, op1=Alu.add,
                    )
                    nc.sync.dma_start(
                        out=y_d[dt * P:(dt + 1) * P, nh * 512:(nh + 1) * 512],
                        in_=y_sb,
                    )

    nc.finalize()
    return nc


def _get_nc():
    if "nc" not in _CACHE:
        _CACHE["nc"] = _build_bass()
    return _CACHE["nc"]


def _make_in_maps(x, gn_w, gn_b, q_w, q_b, k_w, k_b, v_w, v_b, p_w, p_b):
    x = np.asarray(x, np.float32)
    B = x.shape[0]
    assert x.shape == (B, CH, 32, 32) and B == NCORES

    def pc(vec):  # [512] -> [128, 4] with c = t*128 + p
        return np.ascontiguousarray(
            np.asarray(vec, np.float32).reshape(KT, P).T
        )

    shared = {
        "wq_t": np.ascontiguousarray(np.asarray(q_w, np.float32).T),
        "wk_t": np.ascontiguousarray(np.asarray(k_w, np.float32).T),
        "wv_t": np.ascontiguousarray(np.asarray(v_w, np.float32).T),
        "wp_t": np.ascontiguousarray(np.asarray(p_w, np.float32).T),
        "bq_pc": pc(q_b), "bk_pc": pc(k_b), "bv_pc": pc(v_b), "bp_pc": pc(p_b),
        "gnw_pc": pc(gn_w), "gnb_pc": pc(gn_b),
        "ident": np.eye(P, dtype=np.float32),
        "gmask": np.ascontiguousarray(
            np.repeat(np.eye(2, dtype=np.float32), 64, axis=0) / 64.0
        ),
        "bmask": np.ascontiguousarray(
            np.repeat(np.eye(2, dtype=np.float32), 64, axis=0).T
        ),
    }
    return [
        dict(shared, x=np.ascontiguousarray(x[b].reshape(CH, N)))
        for b in range(B)
    ]


def _run(in_maps, **kwargs):
    from concourse.bass_utils import run_bass_kernel_spmd
    return run_bass_kernel_spmd(_get_nc(), in_maps, core_ids=list(range(NCORES)), **kwargs)


def kernel(**inputs):
    in_maps = _make_in_maps(**inputs)
    res = _run(in_maps)
    out = np.stack([r["y"].reshape(CH, 32, 32) for r in res.results], axis=0)
    return out.astype(np.float32)


# revision 6
# speedup vs baseline: 1.0695x; 1.0695x over previous
# Trainium2 Bass kernel for nn_AttentionBlock (GroupNorm + single-head
# self-attention over 32x32 spatial, C=512) — data-parallel over batch:
# 8 batch elements -> 8 NeuronCores, weights replicated.
#
# Self-contained: builds the Bass module lazily, shards the full inputs,
# runs via concourse.bass_utils.run_bass_kernel_spmd, gathers the output.
import numpy as np

CH = 512          # channels
N = 1024          # spatial H*W = 32*32
P = 128           # SBUF partitions
KT = CH // P      # 4 channel tiles
MT = N // P       # 8 spatial tiles (keys)
GROUPS = 8        # groupnorm groups (64 channels each)
EPS = 1e-5
SCALE = 1.0 / np.sqrt(CH)
NCORES = 8

_CACHE = {}


def _build_bass():
    import concourse.bacc as bacc
    import concourse.tile as tile
    from concourse import mybir

    f32 = mybir.dt.float32
    f32r = mybir.dt.float32r
    Act = mybir.ActivationFunctionType
    Alu = mybir.AluOpType

    nc = bacc.Bacc("TRN2")

    x_d = nc.dram_tensor("x", [CH, N], f32, kind="ExternalInput")
    wq_d = nc.dram_tensor("wq_t", [CH, CH], f32, kind="ExternalInput")
    wk_d = nc.dram_tensor("wk_t", [CH, CH], f32, kind="ExternalInput")
    wv_d = nc.dram_tensor("wv_t", [CH, CH], f32, kind="ExternalInput")
    wp_d = nc.dram_tensor("wp_t", [CH, CH], f32, kind="ExternalInput")
    # packed per-channel vectors: cols = bq|bk|bv|bp|gnw|gnb (4 each)
    vec_d = nc.dram_tensor("vecs", [P, 24], f32, kind="ExternalInput")
    ident_d = nc.dram_tensor("ident", [P, P], f32, kind="ExternalInput")
    gmask_d = nc.dram_tensor("gmask", [P, 2], f32, kind="ExternalInput")
    bmask_d = nc.dram_tensor("bmask", [2, P], f32, kind="ExternalInput")
    y_d = nc.dram_tensor("y", [CH, N], f32, kind="ExternalOutput")

    with tile.TileContext(nc) as tc:
        with (
            tc.tile_pool(name="persist", bufs=1) as persist,
            tc.tile_pool(name="work", bufs=2) as work,
            tc.tile_pool(name="small", bufs=2) as small,
            tc.tile_pool(name="ytiles", bufs=3) as ypool,
            tc.tile_pool(name="ps_s", bufs=2, space="PSUM") as ps_s,
            tc.tile_pool(name="ps_t", bufs=2, space="PSUM") as ps_t,
            tc.tile_pool(name="ps_mm", bufs=2, space="PSUM") as ps_mm,
        ):
            # ---- persistent SBUF tensors ----
            x_sb = persist.tile([P, KT, N], f32, tag="x")
            n_sb = persist.tile([P, KT, N], f32r, tag="n")
            q_sb = persist.tile([P, KT, N], f32r, tag="q")
            k_sb = persist.tile([P, KT, N], f32r, tag="k")
            vT_sb = persist.tile([P, MT, CH], f32r, tag="vT")
            aT_sb = persist.tile([P, MT, N], f32r, tag="aT")
            o_sb = persist.tile([P, KT, N], f32r, tag="o")
            wq_sb = persist.tile([P, KT, CH], f32r, tag="wq")
            wk_sb = persist.tile([P, KT, CH], f32r, tag="wk")
            wv_sb = persist.tile([P, KT, CH], f32r, tag="wv")
            wp_sb = persist.tile([P, KT, CH], f32r, tag="wp")
            vec_sb = persist.tile([P, 24], f32, tag="vecs")
            ident_sb = persist.tile([P, P], f32, tag="ident")
            gmask_sb = persist.tile([P, 2], f32, tag="gmask")
            bmask_sb = persist.tile([2, P], f32, tag="bmask")
            zero_sb = persist.tile([P, 1], f32, tag="zero")
            eps_sb = persist.tile([P, 1], f32, tag="eps")
            dummy_sb = persist.tile([P, 1], f32, tag="dummy")

            bq_sb = vec_sb[:, 0:4]
            bk_sb = vec_sb[:, 4:8]
            bv_sb = vec_sb[:, 8:12]
            bp_sb = vec_sb[:, 12:16]
            gnw_sb = vec_sb[:, 16:20]
            gnb_sb = vec_sb[:, 20:24]

            # constants + ACT sqrt-table preload while DMAs stream
            nc.vector.memset(zero_sb, 0.0)
            nc.vector.memset(eps_sb, EPS)
            nc.vector.memset(dummy_sb, 1.0)
            nc.scalar.activation(out=dummy_sb, in_=dummy_sb, func=Act.Sqrt,
                                 bias=eps_sb, scale=1.0)

            # ---- loads ----
            # sync HWDGE: small vectors first (groupnorm needs them), then x
            for v_sb, v_d in ((bmask_sb, bmask_d), (gmask_sb, gmask_d),
                              (vec_sb, vec_d), (ident_sb, ident_d)):
                nc.sync.dma_start(out=v_sb[:], in_=v_d[:])
            for kt in range(KT):
                nc.sync.dma_start(out=x_sb[:, kt, :], in_=x_d[kt * P:(kt + 1) * P, :])
            # q/k weights on the scalar HWDGE queue (needed first), v/p
            # weights on the gpsimd SW DGE — one batched DMA per matrix
            for w_sb, w_d, eng in ((wq_sb, wq_d, nc.scalar), (wk_sb, wk_d, nc.scalar),
                                   (wv_sb, wv_d, nc.gpsimd), (wp_sb, wp_d, nc.gpsimd)):
                src = w_d[:, :].rearrange("(t p) c -> p t c", p=P).bitcast(f32r)
                eng.dma_start(out=w_sb[:], in_=src)

            # ---- GroupNorm, pipelined per channel-tile ----
            # Per-channel mean/E[x^2] (bn_stats on DVE for kt 0/1, activation
            # accumulators on ACT for kt 2/3), then 64-channel group
            # aggregate/broadcast via two tiny mask matmuls per tile.
            for kt in range(KT):
                st = small.tile([P, 2], f32, tag="st")  # mean | E[x^2]
                if kt < 2:
                    bstats = small.tile([P, 2, 6], f32, tag="bstats")
                    mv = small.tile([P, 2], f32, tag="mv")
                    nc.vector.bn_stats(out=bstats[:, 0, :], in_=x_sb[:, kt, 0:512])
                    nc.vector.bn_stats(out=bstats[:, 1, :], in_=x_sb[:, kt, 512:1024])
                    nc.vector.bn_aggr(out=mv, in_=bstats)
                    nc.vector.tensor_copy(st[:, 0:1], mv[:, 0:1])
                    nc.vector.scalar_tensor_tensor(
                        out=st[:, 1:2], in0=mv[:, 0:1], scalar=mv[:, 0:1],
                        in1=mv[:, 1:2], op0=Alu.mult, op1=Alu.add,
                    )
                else:
                    s1 = small.tile([P, 1], f32, tag="s1")
                    s2 = small.tile([P, 1], f32, tag="s2")
                    scratch = work.tile([P, N], f32, tag="scratch")
                    nc.scalar.activation(out=scratch, in_=x_sb[:, kt, :],
                                         func=Act.Identity, bias=zero_sb,
                                         scale=1.0, accum_out=s1)
                    nc.scalar.activation(out=scratch, in_=x_sb[:, kt, :],
                                         func=Act.Square, bias=zero_sb,
                                         scale=1.0, accum_out=s2)
                    nc.vector.tensor_scalar_mul(st[:, 0:1], s1, 1.0 / N)
                    nc.vector.tensor_scalar_mul(st[:, 1:2], s2, 1.0 / N)

                g_ps = ps_mm.tile([2, 2], f32, tag="mm")
                nc.tensor.matmul(g_ps, gmask_sb, st, start=True, stop=True)
                gst = small.tile([2, 2], f32, tag="gst")
                nc.scalar.copy(gst, g_ps)
                b_ps = ps_mm.tile([P, 2], f32, tag="mm")
                nc.tensor.matmul(b_ps, bmask_sb, gst, start=True, stop=True)
                bc = small.tile([P, 2], f32, tag="bc")
                nc.scalar.copy(bc, b_ps)

                mean = bc[:, 0:1]
                var = small.tile([P, 1], f32, tag="var")
                nc.vector.tensor_mul(var, mean, mean)
                nc.vector.tensor_sub(var, bc[:, 1:2], var)  # E[x^2] - mean^2
                tve = small.tile([P, 1], f32, tag="tve")
                nc.vector.tensor_scalar_add(tve, var, EPS)
                sd = small.tile([P, 1], f32, tag="sd")
                nc.scalar.activation(out=sd, in_=var, func=Act.Sqrt,
                                     bias=eps_sb, scale=1.0)
                r0 = small.tile([P, 1], f32, tag="r0")
                nc.vector.reciprocal(r0, sd)
                # one Newton-Raphson step (ACT Sqrt has a loose ULP budget):
                # rstd = r0 * (1.5 - 0.5 * t * r0^2)
                rr = small.tile([P, 1], f32, tag="rr")
                nc.vector.tensor_mul(rr, r0, r0)
                nc.vector.tensor_mul(rr, tve, rr)
                nc.vector.tensor_scalar(rr, rr, -0.5, 1.5, op0=Alu.mult, op1=Alu.add)
                rstd = small.tile([P, 1], f32, tag="rstd")
                nc.vector.tensor_mul(rstd, r0, rr)

                gsc = small.tile([P, 1], f32, tag="gsc")
                gsh = small.tile([P, 1], f32, tag="gsh")
                nc.vector.tensor_mul(gsc, rstd, gnw_sb[:, kt:kt + 1])
                nc.vector.tensor_mul(gsh, mean, gsc)
                nc.vector.tensor_sub(gsh, gnb_sb[:, kt:kt + 1], gsh)
                nc.vector.tensor_scalar(
                    out=n_sb[:, kt, :], in0=x_sb[:, kt, :],
                    scalar1=gsc, scalar2=gsh, op0=Alu.mult, op1=Alu.add,
                )

            # ---- Q, K projections: q[d, n] = sum_c wq[d, c] n[c, n] + bq[d] ----
            for w_sb, b_sb, dst in ((wq_sb, bq_sb, q_sb), (wk_sb, bk_sb, k_sb)):
                for dt in range(KT):
                    for nh in range(2):
                        mm = ps_mm.tile([P, 512], f32, tag="mm")
                        for kt in range(KT):
                            nc.tensor.matmul(
                                mm,
                                w_sb[:, kt, dt * P:(dt + 1) * P],
                                n_sb[:, kt, nh * 512:(nh + 1) * 512],
                                start=(kt == 0), stop=(kt == KT - 1),
                            )
                        nc.scalar.activation(
                            out=dst[:, dt, nh * 512:(nh + 1) * 512], in_=mm,
                            func=Act.Identity, bias=b_sb[:, dt:dt + 1], scale=1.0,
                        )

            # ---- V transposed: vT[m, c] = sum_c' n[c', m] wv_t[c', c] ----
            # (v bias folds into the attention output: rows of attn sum to 1.)
            for mt in range(MT):
                mm = ps_mm.tile([P, 512], f32, tag="mm")
                for kt in range(KT):
                    nc.tensor.matmul(
                        mm,
                        n_sb[:, kt, mt * P:(mt + 1) * P],
                        wv_sb[:, kt, :],
                        start=(kt == 0), stop=(kt == KT - 1),
                    )
                nc.scalar.copy(vT_sb[:, mt, :], mm)

            # ---- attention, software-pipelined over 128-query blocks ----
            # scores -> exp (row sums via the ACT accumulator; softmax max-
            # subtraction dropped: |s*scale| < ~2 so exp is safe and softmax
            # is shift-invariant) -> recip -> diag(1/sum) -> "transpose" via
            # a regular f32r matmul with diag as the moving operand, folding
            # the softmax normalization into the transpose for free.
            def emit_scores(nb):
                s_ps = ps_s.tile([P, N], f32, tag="s", name=f"s{nb}")
                for mh in range(2):
                    for kt in range(KT):
                        nc.tensor.matmul(
                            s_ps[:, mh * 512:(mh + 1) * 512],
                            q_sb[:, kt, nb * P:(nb + 1) * P],
                            k_sb[:, kt, mh * 512:(mh + 1) * 512],
                            start=(kt == 0), stop=(kt == KT - 1),
                        )
                return s_ps

            def emit_softmax(nb, s_ps):
                p_exp = work.tile([P, N], f32r, tag="pexp", name=f"pexp{nb}")
                sumexp = small.tile([P, 1], f32, tag="sumexp")
                nc.scalar.activation(out=p_exp, in_=s_ps, func=Act.Exp,
                                     bias=zero_sb, scale=SCALE, accum_out=sumexp)
                rsum = small.tile([P, 1], f32, tag="rsum")
                nc.vector.reciprocal(rsum, sumexp)
                diag = work.tile([P, P], f32r, tag="diag", name=f"diag{nb}")
                nc.vector.tensor_scalar_mul(diag, ident_sb, rsum)
                return p_exp, diag

            def emit_transposes(nb, p_exp, diag):
                for mg in range(2):
                    t_ps = ps_t.tile([P, 512], f32, tag="t")
                    for mi in range(4):
                        mt = mg * 4 + mi
                        nc.tensor.matmul(
                            t_ps[:, mi * P:(mi + 1) * P],
                            p_exp[:, mt * P:(mt + 1) * P],
                            diag,
                            start=True, stop=True,
                        )
                    nc.vector.tensor_copy(
                        aT_sb[:, mg * 4:(mg + 1) * 4, nb * P:(nb + 1) * P],
                        t_ps.rearrange("p (g c) -> p g c", g=4),
                    )

            pending = None
            for nb in range(MT):
                s_ps = emit_scores(nb)
                sm = emit_softmax(nb, s_ps)
                if pending is not None:
                    emit_transposes(nb - 1, *pending)
                pending = sm
            emit_transposes(MT - 1, *pending)

            # ---- out[c, n] = sum_m vT[m, c] attnT[m, n] (+ bv, folded) ----
            for ct in range(KT):
                for nh in range(2):
                    mm = ps_mm.tile([P, 512], f32, tag="mm")
                    for mt in range(MT):
                        nc.tensor.matmul(
                            mm,
                            vT_sb[:, mt, ct * P:(ct + 1) * P],
                            aT_sb[:, mt, nh * 512:(nh + 1) * 512],
                            start=(mt == 0), stop=(mt == MT - 1),
                        )
                    nc.scalar.activation(
                        out=o_sb[:, ct, nh * 512:(nh + 1) * 512], in_=mm,
                        func=Act.Identity, bias=bv_sb[:, ct:ct + 1], scale=1.0,
                    )

            # ---- final projection + bias + residual, stream out ----
            for dt in range(KT):
                y_sb = ypool.tile([P, N], f32, tag="y")
                for nh in range(2):
                    mm = ps_mm.tile([P, 512], f32, tag="mm")
                    for kt in range(KT):
                        nc.tensor.matmul(
                            mm,
                            wp_sb[:, kt, dt * P:(dt + 1) * P],
                            o_sb[:, kt, nh * 512:(nh + 1) * 512],
                            start=(kt == 0), stop=(kt == KT - 1),
                        )
                    nc.vector.scalar_tensor_tensor(
                        out=y_sb[:, nh * 512:(nh + 1) * 512], in0=mm,
                        scalar=bp_sb[:, dt:dt + 1],
                        in1=x_sb[:, dt, nh * 512:(nh + 1) * 512],
                        op0=Alu.add, op1=Alu.add,
                    )
                nc.sync.dma_start(out=y_d[dt * P:(dt + 1) * P, :], in_=y_sb)

    nc.finalize()
    return nc


def _get_nc():
    if "nc" not in _CACHE:
        _CACHE["nc"] = _build_bass()
    return _CACHE["nc"]


def _make_in_maps(x, gn_w, gn_b, q_w, q_b, k_w, k_b, v_w, v_b, p_w, p_b):
    x = np.asarray(x, np.float32)
    B = x.shape[0]
    assert x.shape == (B, CH, 32, 32) and B == NCORES

    def pc(vec):  # [512] -> [128, 4] with c = t*128 + p
        return np.asarray(vec, np.float32).reshape(KT, P).T

    vecs = np.concatenate(
        [pc(q_b), pc(k_b), pc(v_b), pc(p_b), pc(gn_w), pc(gn_b)], axis=1
    )
    shared = {
        "wq_t": np.ascontiguousarray(np.asarray(q_w, np.float32).T),
        "wk_t": np.ascontiguousarray(np.asarray(k_w, np.float32).T),
        "wv_t": np.ascontiguousarray(np.asarray(v_w, np.float32).T),
        "wp_t": np.ascontiguousarray(np.asarray(p_w, np.float32).T),
        "vecs": np.ascontiguousarray(vecs),
        "ident": np.eye(P, dtype=np.float32),
        "gmask": np.ascontiguousarray(
            np.repeat(np.eye(2, dtype=np.float32), 64, axis=0) / 64.0
        ),
        "bmask": np.ascontiguousarray(
            np.repeat(np.eye(2, dtype=np.float32), 64, axis=0).T
        ),
    }
    return [
        dict(shared, x=np.ascontiguousarray(x[b].reshape(CH, N)))
        for b in range(B)
    ]


def _run(in_maps, **kwargs):
    from concourse.bass_utils import run_bass_kernel_spmd
    return run_bass_kernel_spmd(_get_nc(), in_maps, core_ids=list(range(NCORES)), **kwargs)


def kernel(**inputs):
    in_maps = _make_in_maps(**inputs)
    res = _run(in_maps)
    out = np.stack([r["y"].reshape(CH, 32, 32) for r in res.results], axis=0)
    return out.astype(np.float32)


# revision 8
# speedup vs baseline: 1.2546x; 1.1730x over previous
# Trainium2 Bass kernel for nn_AttentionBlock (GroupNorm + single-head
# self-attention over 32x32 spatial, C=512) — data-parallel over batch:
# 8 batch elements -> 8 NeuronCores, weights replicated.
#
# Self-contained: builds the Bass module lazily, shards the full inputs,
# runs via concourse.bass_utils.run_bass_kernel_spmd, gathers the output.
import numpy as np

CH = 512          # channels
N = 1024          # spatial H*W = 32*32
P = 128           # SBUF partitions
KT = CH // P      # 4 channel tiles
MT = N // P       # 8 spatial tiles (keys)
GROUPS = 8        # groupnorm groups (64 channels each)
EPS = 1e-5
SCALE = 1.0 / np.sqrt(CH)
NCORES = 8

_CACHE = {}


def _build_bass():
    import concourse.bacc as bacc
    import concourse.tile as tile
    from concourse import mybir

    f32 = mybir.dt.float32
    f32r = mybir.dt.float32r
    Act = mybir.ActivationFunctionType
    Alu = mybir.AluOpType

    nc = bacc.Bacc("TRN2")

    x_d = nc.dram_tensor("x", [CH, N], f32, kind="ExternalInput")
    wq_d = nc.dram_tensor("wq_t", [CH, CH], f32, kind="ExternalInput")
    wk_d = nc.dram_tensor("wk_t", [CH, CH], f32, kind="ExternalInput")
    wv_d = nc.dram_tensor("wv_t", [CH, CH], f32, kind="ExternalInput")
    wp_d = nc.dram_tensor("wp_t", [CH, CH], f32, kind="ExternalInput")
    # packed per-channel vectors: cols = bq|bk|bv|bp|gnw|gnb (4 each)
    vec_d = nc.dram_tensor("vecs", [P, 24], f32, kind="ExternalInput")
    # identity (for PE transposes) | block-diag group-averaging matrix
    con_d = nc.dram_tensor("consts", [P, 2, P], f32, kind="ExternalInput")
    y_d = nc.dram_tensor("y", [CH, N], f32, kind="ExternalOutput")

    with tile.TileContext(nc) as tc:
        with (
            tc.tile_pool(name="persist", bufs=1) as persist,
            tc.tile_pool(name="work", bufs=2) as work,
            tc.tile_pool(name="small", bufs=2) as small,
            tc.tile_pool(name="ytiles", bufs=2) as ypool,
        ):
            # ---- persistent SBUF tensors ----
            x_sb = persist.tile([P, KT, N], f32, tag="x")
            n_sb = persist.tile([P, KT, N], f32r, tag="n")
            q_sb = persist.tile([P, KT, N], f32r, tag="q")
            k_sb = persist.tile([P, KT, N], f32r, tag="k")
            vT_sb = persist.tile([P, MT, CH], f32r, tag="vT")
            aT_sb = persist.tile([P, MT, N], f32r, tag="aT")
            o_sb = persist.tile([P, KT, N], f32r, tag="o")
            wq_sb = persist.tile([P, KT, CH], f32r, tag="wq")
            wk_sb = persist.tile([P, KT, CH], f32r, tag="wk")
            wv_sb = persist.tile([P, KT, CH], f32r, tag="wv")
            wp_sb = persist.tile([P, KT, CH], f32r, tag="wp")
            vec_sb = persist.tile([P, 24], f32, tag="vecs")
            ident_sb = persist.tile([P, P], f32r, tag="ident")
            avg_sb = persist.tile([P, P], f32, tag="avg")
            zero_sb = persist.tile([P, 1], f32, tag="zero")
            dummy_sb = persist.tile([P, 1], f32, tag="dummy")
            bq_sb = vec_sb[:, 0:4]
            bk_sb = vec_sb[:, 4:8]
            bv_sb = vec_sb[:, 8:12]
            bp_sb = vec_sb[:, 12:16]
            gnw_sb = vec_sb[:, 16:20]
            gnb_sb = vec_sb[:, 20:24]

            # constants + ACT sqrt-table preload while DMAs stream
            nc.vector.memset(zero_sb, 0.0)
            nc.vector.memset(dummy_sb, 1.0)
            nc.scalar.activation(out=dummy_sb, in_=dummy_sb, func=Act.Sqrt,
                                 bias=zero_sb, scale=1.0)

            # ---- loads: one sync HWDGE queue, ordered by first use so
            # each transfer gets the full HBM bandwidth in sequence ----
            nc.sync.dma_start(out=x_sb[:, 0, :], in_=x_d[0:P, :])
            nc.sync.dma_start(out=vec_sb[:], in_=vec_d[:])
            nc.sync.dma_start(out=ident_sb[:], in_=con_d[:, 0, :].bitcast(f32r))
            nc.sync.dma_start(out=avg_sb[:], in_=con_d[:, 1, :])
            for kt in range(1, KT):
                nc.sync.dma_start(out=x_sb[:, kt, :], in_=x_d[kt * P:(kt + 1) * P, :])
            for w_sb, w_d in ((wq_sb, wq_d), (wk_sb, wk_d),
                              (wv_sb, wv_d), (wp_sb, wp_d)):
                src = w_d[:, :].rearrange("(t p) c -> p t c", p=P).bitcast(f32r)
                nc.sync.dma_start(out=w_sb[:], in_=src)

            with tc.tile_pool(name="ps_a", bufs=2, space="PSUM") as ps_a:
                # ---- GroupNorm, pipelined per channel-tile ----
                # Per-channel mean/E[x^2]: bn_stats on DVE for kt 0/1/3,
                # ACT accumulators for kt 2. Group aggregate+broadcast in a
                # single matmul with a host-built block-averaging matrix.
                for kt in range(KT):
                    st = small.tile([P, 2], f32, tag="st")  # mean | E[x^2]
                    if kt != 2:
                        bstats = small.tile([P, 2, 6], f32, tag="bstats")
                        mv = small.tile([P, 2], f32, tag="mv")
                        nc.vector.bn_stats(out=bstats[:, 0, :], in_=x_sb[:, kt, 0:512])
                        nc.vector.bn_stats(out=bstats[:, 1, :], in_=x_sb[:, kt, 512:1024])
                        nc.vector.bn_aggr(out=mv, in_=bstats)
                        nc.vector.tensor_copy(st[:, 0:1], mv[:, 0:1])
                        nc.vector.scalar_tensor_tensor(
                            out=st[:, 1:2], in0=mv[:, 0:1], scalar=mv[:, 0:1],
                            in1=mv[:, 1:2], op0=Alu.mult, op1=Alu.add,
                        )
                    else:
                        s1 = small.tile([P, 1], f32, tag="s1")
                        s2 = small.tile([P, 1], f32, tag="s2")
                        scratch = work.tile([P, N], f32, tag="scratch")
                        nc.scalar.activation(out=scratch, in_=x_sb[:, kt, :],
                                             func=Act.Identity, bias=zero_sb,
                                             scale=1.0, accum_out=s1)
                        nc.scalar.activation(out=scratch, in_=x_sb[:, kt, :],
                                             func=Act.Square, bias=zero_sb,
                                             scale=1.0, accum_out=s2)
                        nc.vector.tensor_scalar_mul(st[:, 0:1], s1, 1.0 / N)
                        nc.vector.tensor_scalar_mul(st[:, 1:2], s2, 1.0 / N)

                    # group stats broadcast to all 128 partitions: one matmul
                    b_ps = ps_a.tile([P, 2], f32, tag="mm")
                    nc.tensor.matmul(b_ps, avg_sb, st, start=True, stop=True)
                    bc = small.tile([P, 2], f32, tag="bc")
                    nc.scalar.copy(bc, b_ps)

                    mean = bc[:, 0:1]
                    # tve = (E[x^2] - mean^2) + eps, via negated-variance
                    vneg = small.tile([P, 1], f32, tag="vneg")
                    nc.vector.scalar_tensor_tensor(
                        out=vneg, in0=mean, scalar=mean, in1=bc[:, 1:2],
                        op0=Alu.mult, op1=Alu.subtract,  # mean^2 - E[x^2]
                    )
                    tve = small.tile([P, 1], f32, tag="tve")
                    nc.vector.tensor_scalar(tve, vneg, -1.0, EPS,
                                            op0=Alu.mult, op1=Alu.add)
                    sd = small.tile([P, 1], f32, tag="sd")
                    nc.scalar.activation(out=sd, in_=tve, func=Act.Sqrt,
                                         bias=zero_sb, scale=1.0)
                    r0 = small.tile([P, 1], f32, tag="r0")
                    nc.vector.reciprocal(r0, sd)
                    # one Newton step (ACT Sqrt has a loose ULP budget):
                    # rstd = r0 * (1.5 - 0.5 * tve * r0^2)
                    rr = small.tile([P, 1], f32, tag="rr")
                    nc.vector.scalar_tensor_tensor(
                        out=rr, in0=r0, scalar=r0, in1=tve,
                        op0=Alu.mult, op1=Alu.mult,
                    )
                    nc.vector.tensor_scalar(rr, rr, -0.5, 1.5,
                                            op0=Alu.mult, op1=Alu.add)
                    rstd = small.tile([P, 1], f32, tag="rstd")
                    nc.vector.tensor_mul(rstd, r0, rr)

                    gsc = small.tile([P, 1], f32, tag="gsc")
                    nc.vector.tensor_mul(gsc, rstd, gnw_sb[:, kt:kt + 1])
                    gshn = small.tile([P, 1], f32, tag="gshn")  # -shift
                    nc.vector.scalar_tensor_tensor(
                        out=gshn, in0=mean, scalar=gsc, in1=gnb_sb[:, kt:kt + 1],
                        op0=Alu.mult, op1=Alu.subtract,
                    )
                    nc.vector.tensor_scalar(
                        out=n_sb[:, kt, :], in0=x_sb[:, kt, :],
                        scalar1=gsc, scalar2=gshn, op0=Alu.mult, op1=Alu.subtract,
                    )

                # preload the exp table while the QKV matmuls stream
                nc.scalar.activation(out=dummy_sb, in_=dummy_sb, func=Act.Exp,
                                     bias=zero_sb, scale=1.0)

                # ---- Q, K projections ----
                for w_sb, b_sb, dst in ((wq_sb, bq_sb, q_sb), (wk_sb, bk_sb, k_sb)):
                    for dt in range(KT):
                        for nh in range(2):
                            mm = ps_a.tile([P, 512], f32, tag="mm")
                            for kt in range(KT):
                                nc.tensor.matmul(
                                    mm,
                                    w_sb[:, kt, dt * P:(dt + 1) * P],
                                    n_sb[:, kt, nh * 512:(nh + 1) * 512],
                                    start=(kt == 0), stop=(kt == KT - 1),
                                )
                            nc.scalar.activation(
                                out=dst[:, dt, nh * 512:(nh + 1) * 512], in_=mm,
                                func=Act.Identity, bias=b_sb[:, dt:dt + 1], scale=1.0,
                            )

                # ---- V transposed: vT[m, c] = sum_c' n[c', m] wv_t[c', c] ----
                # (v bias folds into the attention output: attn rows sum to 1)
                for mt in range(MT):
                    mm = ps_a.tile([P, 512], f32, tag="mm")
                    for kt in range(KT):
                        nc.tensor.matmul(
                            mm,
                            n_sb[:, kt, mt * P:(mt + 1) * P],
                            wv_sb[:, kt, :],
                            start=(kt == 0), stop=(kt == KT - 1),
                        )
                    nc.scalar.copy(vT_sb[:, mt, :], mm)

            # ---- attention, software-pipelined over 128-query blocks ----
            # scores -> exp (row sums via ACT accumulator; max-subtraction
            # dropped: |s*scale| < ~2 so exp is safe and softmax is
            # shift-invariant) -> normalize in place -> PE transpose-mode
            # (f32r: 1.5 cyc/row). Two score blocks run ahead of the
            # softmax/transpose of the previous block to keep PE dense.
            with (
                tc.tile_pool(name="ps_s", bufs=3, space="PSUM") as ps_s,
                tc.tile_pool(name="ps_t", bufs=2, space="PSUM") as ps_t,
            ):
                def emit_scores(nb):
                    s_ps = ps_s.tile([P, N], f32, tag="s", name=f"s{nb}")
                    for mh in range(2):
                        for kt in range(KT):
                            nc.tensor.matmul(
                                s_ps[:, mh * 512:(mh + 1) * 512],
                                q_sb[:, kt, nb * P:(nb + 1) * P],
                                k_sb[:, kt, mh * 512:(mh + 1) * 512],
                                start=(kt == 0), stop=(kt == KT - 1),
                            )
                    return s_ps

                def emit_softmax(nb, s_ps):
                    p_exp = work.tile([P, N], f32r, tag="pexp", name=f"pexp{nb}")
                    sumexp = small.tile([P, 1], f32, tag="sumexp")
                    nc.scalar.activation(out=p_exp, in_=s_ps, func=Act.Exp,
                                         bias=zero_sb, scale=SCALE,
                                         accum_out=sumexp)
                    rsum = small.tile([P, 1], f32, tag="rsum")
                    nc.vector.reciprocal(rsum, sumexp)
                    nc.vector.tensor_scalar_mul(p_exp, p_exp, rsum)
                    return p_exp

                def emit_transposes(nb, p_exp):
                    for mg in range(2):
                        t_ps = ps_t.tile([P, 512], f32r, tag="t")
                        for mi in range(4):
                            mt = mg * 4 + mi
                            nc.tensor.transpose(
                                t_ps[:, mi * P:(mi + 1) * P],
                                p_exp[:, mt * P:(mt + 1) * P],
                                ident_sb[:],
                            )
                        nc.vector.tensor_copy(
                            aT_sb[:, mg * 4:(mg + 1) * 4, nb * P:(nb + 1) * P],
                            t_ps.rearrange("p (g c) -> p g c", g=4),
                        )

                pipeline = []
                for nb in range(MT):
                    s_ps = emit_scores(nb)
                    pe = emit_softmax(nb, s_ps)
                    pipeline.append((nb, pe))
                    if len(pipeline) > 2:
                        emit_transposes(*pipeline.pop(0))
                for item in pipeline:
                    emit_transposes(*item)

            with tc.tile_pool(name="ps_b", bufs=4, space="PSUM") as ps_b:
                # ---- out[c, n] = sum_m vT[m, c] attnT[m, n] (+ bv, folded) ----
                for ct in range(KT):
                    for nh in range(2):
                        mm = ps_b.tile([P, 512], f32, tag="mm")
                        for mt in range(MT):
                            nc.tensor.matmul(
                                mm,
                                vT_sb[:, mt, ct * P:(ct + 1) * P],
                                aT_sb[:, mt, nh * 512:(nh + 1) * 512],
                                start=(mt == 0), stop=(mt == MT - 1),
                            )
                        nc.scalar.activation(
                            out=o_sb[:, ct, nh * 512:(nh + 1) * 512], in_=mm,
                            func=Act.Identity, bias=bv_sb[:, ct:ct + 1], scale=1.0,
                        )

                # ---- final projection + bias + residual, stream out ----
                for dt in range(KT):
                    y_sb = ypool.tile([P, N], f32, tag="y")
                    for nh in range(2):
                        mm = ps_b.tile([P, 512], f32, tag="mm")
                        for kt in range(KT):
                            nc.tensor.matmul(
                                mm,
                                wp_sb[:, kt, dt * P:(dt + 1) * P],
                                o_sb[:, kt, nh * 512:(nh + 1) * 512],
                                start=(kt == 0), stop=(kt == KT - 1),
                            )
                        nc.vector.scalar_tensor_tensor(
                            out=y_sb[:, nh * 512:(nh + 1) * 512], in0=mm,
                            scalar=bp_sb[:, dt:dt + 1],
                            in1=x_sb[:, dt, nh * 512:(nh + 1) * 512],
                            op0=Alu.add, op1=Alu.add,
                        )
                    nc.sync.dma_start(out=y_d[dt * P:(dt + 1) * P, :], in_=y_sb)

    nc.finalize()
    return nc


def _get_nc():
    if "nc" not in _CACHE:
        _CACHE["nc"] = _build_bass()
    return _CACHE["nc"]


def _make_in_maps(x, gn_w, gn_b, q_w, q_b, k_w, k_b, v_w, v_b, p_w, p_b):
    x = np.asarray(x, np.float32)
    B = x.shape[0]
    assert x.shape == (B, CH, 32, 32) and B == NCORES

    def pc(vec):  # [512] -> [128, 4] with c = t*128 + p
        return np.asarray(vec, np.float32).reshape(KT, P).T

    vecs = np.concatenate(
        [pc(q_b), pc(k_b), pc(v_b), pc(p_b), pc(gn_w), pc(gn_b)], axis=1
    )
    # identity + block-diagonal 64-channel averaging matrix, stacked
    avg = np.kron(np.eye(2, dtype=np.float32),
                  np.full((64, 64), 1.0 / 64, np.float32))
    consts = np.stack([np.eye(P, dtype=np.float32), avg], axis=1)
    shared = {
        "wq_t": np.ascontiguousarray(np.asarray(q_w, np.float32).T),
        "wk_t": np.ascontiguousarray(np.asarray(k_w, np.float32).T),
        "wv_t": np.ascontiguousarray(np.asarray(v_w, np.float32).T),
        "wp_t": np.ascontiguousarray(np.asarray(p_w, np.float32).T),
        "vecs": np.ascontiguousarray(vecs),
        "consts": np.ascontiguousarray(consts),
    }
    return [
        dict(shared, x=np.ascontiguousarray(x[b].reshape(CH, N)))
        for b in range(B)
    ]


def _run(in_maps, **kwargs):
    from concourse.bass_utils import run_bass_kernel_spmd
    return run_bass_kernel_spmd(_get_nc(), in_maps, core_ids=list(range(NCORES)), **kwargs)


def kernel(**inputs):
    in_maps = _make_in_maps(**inputs)
    res = _run(in_maps)
    out = np.stack([r["y"].reshape(CH, 32, 32) for r in res.results], axis=0)
    return out.astype(np.float32)
